# revision 31
# baseline (speedup 1.0000x reference)
"""Blockwise 8x8 2D DCT (ortho DCT-II) on Trainium2, 8 NeuronCores data-parallel.

Per 8x8 block: Y = A @ X @ A.T, with M = kron(I_16, A) acting on 128-row tiles.

Key trick ("fused" op): a regular PE matmul with the DATA as the stationary
operand computes  out = chunk^T @ M^T = (M @ chunk)^T  — one DCT pass plus a
128x128 transpose in a single instruction. Two fused passes give
  pass1: (M X)^T   (W-major)     pass2: ((M X) M^T)  (back to H-major)
In bf16 the stationary load gets FWL (2x), so each fused op is ~LDW+128 cols.

Modes:
  fused_bf16  : cast x->bf16 at DMA load (gpsimd SWDGE cast); both passes fused bf16.
  hybrid      : V-pass as f32r streaming matmul (x stays fp32-exact), bf16
                transposes, fused bf16 H-pass.
  stream_f32r : f32r streaming matmuls + f32r PE transposes both directions.
"""

import numpy as np
import ml_dtypes

import concourse.bass as bass
import concourse.bacc as bacc
import concourse.mybir as mybir
from concourse import tile
from concourse.bass_utils import run_bass_kernel_spmd

F32 = mybir.dt.float32
F32R = mybir.dt.float32r
BF16 = mybir.dt.bfloat16
FP16 = mybir.dt.float16
P = 128
BLOCK = 8
N_CORES = 8

FULL_N, FULL_C, FULL_H, FULL_W = 64, 1, 1024, 1024

MODE = "v2"


class _CopyBalancer:
    """Deterministically split PSUM->SBUF copies between DVE and ACT."""

    def __init__(self, nc, dve_of_8=5):
        self.nc = nc
        self.k = dve_of_8
        self.i = 0

    def copy(self, out, in_):
        if self.i % 8 < self.k:
            self.nc.vector.tensor_copy(out, in_)
        else:
            self.nc.scalar.copy(out, in_)
        self.i += 1


def build_fused_bf16(n_img: int, img_h: int, width: int, dt16=BF16):
    rows = n_img * img_h
    nrt, nwt = img_h // P, width // P
    assert nrt % 4 == 0 and nwt % 4 == 0

    nc = bacc.Bacc("TRN2", target_bir_lowering=False, debug=False)
    x_d = nc.declare_dram_parameter("x", [rows, width], F32, isOutput=False)
    mtb_d = nc.declare_dram_parameter("mtb", [P, P], dt16, isOutput=False)
    out_d = nc.declare_dram_parameter("out", [rows, width], F32, isOutput=True)

    with tile.TileContext(nc) as tc:
        with (
            tc.tile_pool(name="consts", bufs=1) as cpool,
            tc.tile_pool(name="xin", bufs=nrt + 4) as xpool,
            tc.tile_pool(name="y1t", bufs=nwt + 4) as y1tpool,
            tc.tile_pool(name="outp", bufs=4) as outpool,
            tc.tile_pool(name="psA", bufs=4, space="PSUM") as psA,
            tc.tile_pool(name="psB", bufs=4, space="PSUM") as psB,
        ):
            cb = _CopyBalancer(nc)
            mtb_sb = cpool.tile([P, P], dt16)
            nc.sync.dma_start(mtb_sb[:], mtb_d[:])

            for img in range(n_img):
                r0 = img * img_h

                xts = []
                for rt in range(nrt):
                    xt = xpool.tile([P, width], dt16)
                    # SWDGE cast f32 -> f16 during the load; halves ramp faster
                    for h in range(2):
                        cols = slice(h * width // 2, (h + 1) * width // 2)
                        nc.gpsimd.dma_start(
                            xt[:, cols],
                            x_d[r0 + rt * P : r0 + (rt + 1) * P, cols],
                        )
                    xts.append(xt)

                # pass 1: y1t[wt][:, rt*128:+128] = (M @ x_chunk)^T
                y1ts = []
                for wt in range(nwt):
                    y1t = y1tpool.tile([P, img_h], dt16)
                    for half in range(nrt // 4):
                        ps = psA.tile([P, 512], F32)
                        for q in range(4):
                            rt = half * 4 + q
                            nc.tensor.matmul(
                                ps[:, q * P : (q + 1) * P],
                                xts[rt][:, wt * P : (wt + 1) * P],
                                mtb_sb[:],
                            )
                        cb.copy(y1t[:, half * 512 : (half + 1) * 512], ps[:])
                    y1ts.append(y1t)

                # pass 2: out[rt][:, wt*128:+128] = (M @ y1t_chunk)^T = final
                for rt in range(nrt):
                    out_sb = outpool.tile([P, width], F32)
                    for half in range(nwt // 4):
                        ps = psB.tile([P, 512], F32)
                        for q in range(4):
                            wt = half * 4 + q
                            nc.tensor.matmul(
                                ps[:, q * P : (q + 1) * P],
                                y1ts[wt][:, rt * P : (rt + 1) * P],
                                mtb_sb[:],
                            )
                        cb.copy(out_sb[:, half * 512 : (half + 1) * 512], ps[:])
                        # store each 512-col half as soon as it lands
                        nc.sync.dma_start(
                            out_d[
                                r0 + rt * P : r0 + (rt + 1) * P,
                                half * 512 : (half + 1) * 512,
                            ],
                            out_sb[:, half * 512 : (half + 1) * 512],
                        )

    nc.compile()
    return nc


def build_v2(n_img: int, img_h: int, width: int, dt16=FP16, psum_fd=512,
             act_of_4=2, ps_bufs=8, store_eng="scalar", x_bufs=None):
    """fp16 DRAM in/out (host casts both ways): halves HBM traffic vs f32.

    HWDGE loads on the sync ring, stores on the scalar ring (separate FIFOs).
    Matmuls accumulate f32 into PSUM tiles of [128, psum_fd]; the PSUM->SBUF
    copy casts to fp16 and is split between ACT and DVE (act_of_4 of 4).
    """
    rows = n_img * img_h
    nrt, nwt = img_h // P, width // P
    mm_per_ps = psum_fd // P

    nc = bacc.Bacc("TRN2", target_bir_lowering=False, debug=False)
    x_d = nc.declare_dram_parameter("x", [rows, width], dt16, isOutput=False)
    mtb_d = nc.declare_dram_parameter("mtb", [P, P], dt16, isOutput=False)
    out_d = nc.declare_dram_parameter("out", [rows, width], dt16, isOutput=True)

    if x_bufs is None:
        x_bufs = nrt + 4

    with tile.TileContext(nc) as tc:
        with (
            tc.tile_pool(name="consts", bufs=1) as cpool,
            tc.tile_pool(name="xin", bufs=x_bufs) as xpool,
            tc.tile_pool(name="y1t", bufs=nwt + 4) as y1tpool,
            tc.tile_pool(name="outp", bufs=6) as outpool,
            tc.tile_pool(name="ps", bufs=ps_bufs, space="PSUM") as psp,
        ):
            store_engine = getattr(nc, store_eng)
            copy_i = 0

            def cb_copy(out, in_):
                nonlocal copy_i
                if copy_i % 4 < act_of_4:
                    nc.scalar.copy(out, in_)
                else:
                    nc.vector.tensor_copy(out, in_)
                copy_i += 1

            mtb_sb = cpool.tile([P, P], dt16)
            nc.sync.dma_start(mtb_sb[:], mtb_d[:])

            for img in range(n_img):
                r0 = img * img_h

                xts = []
                for rt in range(nrt):
                    xt = xpool.tile([P, width], dt16)
                    nc.sync.dma_start(
                        xt[:], x_d[r0 + rt * P : r0 + (rt + 1) * P, :]
                    )
                    xts.append(xt)

                # pass 1: y1t[wt][:, rt*128:+128] = (M @ x_chunk)^T
                y1ts = []
                for wt in range(nwt):
                    y1t = y1tpool.tile([P, img_h], dt16)
                    for g in range(nrt // mm_per_ps):
                        ps = psp.tile([P, psum_fd], F32)
                        for q in range(mm_per_ps):
                            rt = g * mm_per_ps + q
                            nc.tensor.matmul(
                                ps[:, q * P : (q + 1) * P],
                                xts[rt][:, wt * P : (wt + 1) * P],
                                mtb_sb[:],
                            )
                        cb_copy(
                            y1t[:, g * psum_fd : (g + 1) * psum_fd], ps[:]
                        )
                    y1ts.append(y1t)

                # pass 2: out[rt][:, wt*128:+128] = (M @ y1t_chunk)^T = final
                for rt in range(nrt):
                    out_sb = outpool.tile([P, width], dt16)
                    for g in range(nwt // mm_per_ps):
                        ps = psp.tile([P, psum_fd], F32)
                        for q in range(mm_per_ps):
                            wt = g * mm_per_ps + q
                            nc.tensor.matmul(
                                ps[:, q * P : (q + 1) * P],
                                y1ts[wt][:, rt * P : (rt + 1) * P],
                                mtb_sb[:],
                            )
                        cb_copy(
                            out_sb[:, g * psum_fd : (g + 1) * psum_fd], ps[:]
                        )
                    store_engine.dma_start(
                        out_d[r0 + rt * P : r0 + (rt + 1) * P, :], out_sb[:]
                    )

    nc.compile()
    return nc


def build_v4(n_img: int, img_h: int, width: int, dt16=FP16, psum_fd=1024,
             act_of_4=2, ps_bufs=4, x_bufs=20, y1t_bufs=18, out_bufs=6,
             warm_mm=48, load_gran=1, store_gran=1):
    """v3 + software pipelining: pass2 of image i-1 interleaves with pass1 of
    image i at PSUM-group granularity, so pass2 stationaries (y1t copies)
    have a full image-period of slack. A dummy-matmul warmup burst runs
    during the first image's load so HAM un-throttles before real work.
    """
    rows = n_img * img_h
    nrt, nwt = img_h // P, width // P
    mm_per_ps = psum_fd // P

    nc = bacc.Bacc("TRN2", target_bir_lowering=False, debug=False)
    x_d = nc.declare_dram_parameter("x", [rows, width], dt16, isOutput=False)
    mtb_d = nc.declare_dram_parameter("mtb", [P, P], dt16, isOutput=False)
    out_d = nc.declare_dram_parameter("out", [rows, width], dt16, isOutput=True)

    with tile.TileContext(nc) as tc:
        with (
            tc.tile_pool(name="consts", bufs=1) as cpool,
            tc.tile_pool(name="xin", bufs=x_bufs) as xpool,
            tc.tile_pool(name="y1t", bufs=y1t_bufs) as y1tpool,
            tc.tile_pool(name="outp", bufs=out_bufs) as outpool,
            tc.tile_pool(name="ps", bufs=ps_bufs, space="PSUM") as psp,
        ):
            copy_i = 0

            def cb_copy(out, in_):
                nonlocal copy_i
                if copy_i % 4 < act_of_4:
                    nc.scalar.copy(out, in_)
                else:
                    nc.vector.tensor_copy(out, in_)
                copy_i += 1

            mtb_sb = cpool.tile([P, P], dt16)
            nc.sync.dma_start(mtb_sb[:], mtb_d[:])

            xts_of = {}
            y1ts_of = {}

            strip_w = width // load_gran

            def load_img(i):
                # column-strip granules: strip s holds cols [s*strip_w,
                # (s+1)*strip_w) of every row-tile, so pass1 group wt only
                # depends on strip wt*P//strip_w instead of the whole image
                r0 = i * img_h
                xg = []
                for s in range(load_gran):
                    xt = xpool.tile([P, nrt, strip_w], dt16)
                    src = x_d[
                        r0 : r0 + img_h, s * strip_w : (s + 1) * strip_w
                    ].rearrange("(t p) w -> p t w", p=P)
                    nc.sync.dma_start(xt[:], src)
                    xg.append(xt)
                xts_of[i] = xg

            def x_chunk(i, rt, wt):
                xt = xts_of[i][(wt * P) // strip_w]
                return xt[:, rt, (wt * P) % strip_w : (wt * P) % strip_w + P]

            def p1_group(i, wt):
                # y1t(i)[wt][:, rt*128:+128] = (M @ x_chunk)^T for all rt
                y1t = y1tpool.tile([P, img_h], dt16)
                for g in range(nrt // mm_per_ps):
                    ps = psp.tile([P, psum_fd], F32)
                    for q in range(mm_per_ps):
                        rt = g * mm_per_ps + q
                        nc.tensor.matmul(
                            ps[:, q * P : (q + 1) * P],
                            x_chunk(i, rt, wt),
                            mtb_sb[:],
                        )
                    cb_copy(y1t[:, g * psum_fd : (g + 1) * psum_fd], ps[:])
                y1ts_of.setdefault(i, []).append(y1t)

            out_cur = {}

            def p2_group(i, rt, half=None):
                # out(i)[rt][:, wt*128:+128] = (M @ y1t_chunk)^T
                # half=0/1: emit only the wt-half (4 matmuls + 1 copy); half 0
                # needs only y1t[0..3], so it can start before pass1 finishes
                r0 = i * img_h
                y1ts = y1ts_of[i]
                seg = rt % store_gran
                pair = rt - seg
                if pair not in out_cur:
                    out_cur[pair] = outpool.tile(
                        [P, store_gran, width], dt16, name="out_sb"
                    )
                out_sb = out_cur[pair]
                if half is None:
                    ps = psp.tile([P, psum_fd], F32)
                    for q in range(mm_per_ps):
                        nc.tensor.matmul(
                            ps[:, q * P : (q + 1) * P],
                            y1ts[q][:, rt * P : (rt + 1) * P],
                            mtb_sb[:],
                        )
                    cb_copy(out_sb[:, seg, :], ps[:])
                else:
                    ps = psp.tile([P, psum_fd], F32)
                    fd = psum_fd // 2
                    for q in range(fd // P):
                        wt = half * (fd // P) + q
                        nc.tensor.matmul(
                            ps[:, q * P : (q + 1) * P],
                            y1ts[wt][:, rt * P : (rt + 1) * P],
                            mtb_sb[:],
                        )
                    cb_copy(
                        out_sb[:, seg, half * fd : (half + 1) * fd], ps[:, :fd]
                    )
                if (half is None or half == 1) and seg == store_gran - 1:
                    del out_cur[pair]
                    dst = out_d[
                        r0 + pair * P : r0 + (pair + store_gran) * P, :
                    ].rearrange("(t p) w -> p t w", p=P)
                    nc.gpsimd.dma_start(dst, out_sb[:])

            # HAM warmup: dummy matmuls on the constant while image 0 loads.
            load_img(0)
            for w in range(warm_mm // mm_per_ps):
                ps = psp.tile([P, psum_fd], F32)
                for q in range(mm_per_ps):
                    nc.tensor.matmul(
                        ps[:, q * P : (q + 1) * P], mtb_sb[:], mtb_sb[:]
                    )

            for i in range(n_img):
                if i + 1 < n_img:
                    load_img(i + 1)
                for g in range(nwt):
                    p1_group(i, g)
                    if i > 0:
                        p2_group(i - 1, g)
            for g in range(nwt):
                p2_group(n_img - 1, g)

    nc.compile()
    return nc


def build_hybrid(n_img: int, img_h: int, width: int):
    rows = n_img * img_h
    nrt, nwt = img_h // P, width // P
    assert nrt % 4 == 0 and nwt % 4 == 0
    MMW = 512

    nc = bacc.Bacc("TRN2", target_bir_lowering=False, debug=False)
    x_d = nc.declare_dram_parameter("x", [rows, width], F32R, isOutput=False)
    mt_d = nc.declare_dram_parameter("mt", [P, P], F32R, isOutput=False)
    mtb_d = nc.declare_dram_parameter("mtb", [P, P], BF16, isOutput=False)
    identb_d = nc.declare_dram_parameter("identb", [P, P], BF16, isOutput=False)
    out_d = nc.declare_dram_parameter("out", [rows, width], F32, isOutput=True)

    with tile.TileContext(nc) as tc:
        with (
            tc.tile_pool(name="consts", bufs=1) as cpool,
            tc.tile_pool(name="xin", bufs=6) as xpool,
            tc.tile_pool(name="y1", bufs=nrt + 2) as y1pool,
            tc.tile_pool(name="y1t", bufs=nwt + 4) as y1tpool,
            tc.tile_pool(name="outp", bufs=4) as outpool,
            tc.tile_pool(name="psV", bufs=3, space="PSUM") as psV,
            tc.tile_pool(name="psT", bufs=3, space="PSUM") as psT,
            tc.tile_pool(name="psH", bufs=2, space="PSUM") as psH,
        ):
            cb = _CopyBalancer(nc)
            mt_sb = cpool.tile([P, P], F32R)
            mtb_sb = cpool.tile([P, P], BF16)
            identb = cpool.tile([P, P], BF16)
            nc.sync.dma_start(mt_sb[:], mt_d[:])
            nc.sync.dma_start(mtb_sb[:], mtb_d[:])
            nc.sync.dma_start(identb[:], identb_d[:])

            for img in range(n_img):
                r0 = img * img_h

                # V-pass: f32r stream, round to bf16 on the PSUM->SBUF copy
                y1s = []
                for rt in range(nrt):
                    xt = xpool.tile([P, width], F32R)
                    nc.sync.dma_start(
                        xt[:], x_d[r0 + rt * P : r0 + (rt + 1) * P, :]
                    )
                    y1 = y1pool.tile([P, width], BF16)
                    for c in range(width // MMW):
                        ps = psV.tile([P, MMW], F32)
                        nc.tensor.matmul(
                            ps[:], mt_sb[:], xt[:, c * MMW : (c + 1) * MMW]
                        )
                        cb.copy(y1[:, c * MMW : (c + 1) * MMW], ps[:])
                    y1s.append(y1)

                # T-pass: bf16 PE transposes, 8 per PSUM bank
                y1ts = []
                for wt in range(nwt):
                    y1t = y1tpool.tile([P, img_h], BF16)
                    pst = psT.tile([P, img_h], BF16)
                    for rt in range(nrt):
                        nc.tensor.transpose(
                            pst[:, rt * P : (rt + 1) * P],
                            y1s[rt][:, wt * P : (wt + 1) * P],
                            identb[:],
                        )
                    cb.copy(y1t[:], pst[:])
                    y1ts.append(y1t)

                # fused H-pass: out chunk = (y1t_chunk)^T @ M^T  (H-major)
                for rt in range(nrt):
                    out_sb = outpool.tile([P, width], F32)
                    for half in range(nwt // 4):
                        ps = psH.tile([P, 512], F32)
                        for q in range(4):
                            wt = half * 4 + q
                            nc.tensor.matmul(
                                ps[:, q * P : (q + 1) * P],
                                y1ts[wt][:, rt * P : (rt + 1) * P],
                                mtb_sb[:],
                            )
                        cb.copy(out_sb[:, half * 512 : (half + 1) * 512], ps[:])
                    nc.sync.dma_start(
                        out_d[r0 + rt * P : r0 + (rt + 1) * P, :], out_sb[:]
                    )

    nc.compile()
    return nc


def build_nc(n_img, img_h, width, mode=MODE):
    if mode == "fused_bf16":
        return build_fused_bf16(n_img, img_h, width, BF16)
    if mode == "fused_fp16":
        return build_fused_bf16(n_img, img_h, width, FP16)
    if mode == "hybrid":
        return build_hybrid(n_img, img_h, width)
    if mode == "v2":
        return build_v2(n_img, img_h, width)
    if mode == "v3":
        return build_v2(n_img, img_h, width, psum_fd=1024, ps_bufs=4,
                        store_eng="gpsimd", x_bufs=20)
    if mode == "v4":
        return build_v4(n_img, img_h, width)
    if mode == "v6":
        return build_v4(n_img, img_h, width, load_gran=4, store_gran=2,
                        x_bufs=5, out_bufs=4)
    raise ValueError(mode)


def make_mt(A: np.ndarray) -> np.ndarray:
    """M^T where M = kron(I_{128/8}, A)."""
    M = np.kron(np.eye(P // BLOCK, dtype=np.float32), A.astype(np.float32))
    return np.ascontiguousarray(M.T)


def make_inputs(mode, x_core, A):
    mt = make_mt(A)
    if mode == "fused_bf16":
        return {"x": x_core, "mtb": mt.astype(ml_dtypes.bfloat16)}
    if mode == "fused_fp16":
        return {"x": x_core, "mtb": mt.astype(np.float16)}
    if mode[0] == "v" and mode[1].isdigit() and int(mode[1]) >= 2:
        return {"x": x_core.astype(np.float16), "mtb": mt.astype(np.float16)}
    if mode == "hybrid":
        return {
            "x": x_core,
            "mt": mt,
            "mtb": mt.astype(ml_dtypes.bfloat16),
            "identb": np.eye(P, dtype=ml_dtypes.bfloat16),
        }
    raise ValueError(mode)


_NC_CACHE = {}


def _get_nc(key, *args, **kwargs):
    if key not in _NC_CACHE:
        _NC_CACHE[key] = build_nc(*args, **kwargs)
    return _NC_CACHE[key]


def kernel(x: np.ndarray, A: np.ndarray) -> np.ndarray:
    x = np.asarray(x, dtype=np.float32)
    A = np.asarray(A, dtype=np.float32)
    N, C, H, W = x.shape
    assert (N, C, H, W) == (FULL_N, FULL_C, FULL_H, FULL_W), x.shape
    per = N // N_CORES

    nc = _get_nc(("full", MODE), per * C, H, W, MODE)

    in_maps = [
        make_inputs(
            MODE,
            np.ascontiguousarray(x[c * per : (c + 1) * per].reshape(per * C * H, W)),
            A,
        )
        for c in range(N_CORES)
    ]
    last_err = None
    for _attempt in range(3):
        try:
            res = run_bass_kernel_spmd(nc, in_maps, list(range(N_CORES)))
            break
        except Exception as e:  # transient NRT device faults: retry
            last_err = e
    else:
        raise last_err
    outs = [
        res.results[c]["out"].astype(np.float32, copy=False).reshape(per, C, H, W)
        for c in range(N_CORES)
    ]
    return np.concatenate(outs, axis=0)



# revision 34
# speedup vs baseline: 1.1161x; 1.1161x over previous
"""Blockwise 8x8 2D DCT (ortho DCT-II) on Trainium2, 8 NeuronCores data-parallel.

Per 8x8 block: Y = A @ X @ A.T, with M = kron(I_16, A) acting on 128-row tiles.

Key trick ("fused" op): a regular PE matmul with the DATA as the stationary
operand computes  out = chunk^T @ M^T = (M @ chunk)^T  — one DCT pass plus a
128x128 transpose in a single instruction. Two fused passes give
  pass1: (M X)^T   (W-major)     pass2: ((M X) M^T)  (back to H-major)
In bf16 the stationary load gets FWL (2x), so each fused op is ~LDW+128 cols.

Modes:
  fused_bf16  : cast x->bf16 at DMA load (gpsimd SWDGE cast); both passes fused bf16.
  hybrid      : V-pass as f32r streaming matmul (x stays fp32-exact), bf16
                transposes, fused bf16 H-pass.
  stream_f32r : f32r streaming matmuls + f32r PE transposes both directions.
"""

import numpy as np
import ml_dtypes

import concourse.bass as bass
import concourse.bacc as bacc
import concourse.mybir as mybir
from concourse import tile
from concourse.bass_utils import run_bass_kernel_spmd

F32 = mybir.dt.float32
F32R = mybir.dt.float32r
BF16 = mybir.dt.bfloat16
FP16 = mybir.dt.float16
P = 128
BLOCK = 8
N_CORES = 8

FULL_N, FULL_C, FULL_H, FULL_W = 64, 1, 1024, 1024

MODE = "v6"


class _CopyBalancer:
    """Deterministically split PSUM->SBUF copies between DVE and ACT."""

    def __init__(self, nc, dve_of_8=5):
        self.nc = nc
        self.k = dve_of_8
        self.i = 0

    def copy(self, out, in_):
        if self.i % 8 < self.k:
            self.nc.vector.tensor_copy(out, in_)
        else:
            self.nc.scalar.copy(out, in_)
        self.i += 1


def build_fused_bf16(n_img: int, img_h: int, width: int, dt16=BF16):
    rows = n_img * img_h
    nrt, nwt = img_h // P, width // P
    assert nrt % 4 == 0 and nwt % 4 == 0

    nc = bacc.Bacc("TRN2", target_bir_lowering=False, debug=False)
    x_d = nc.declare_dram_parameter("x", [rows, width], F32, isOutput=False)
    mtb_d = nc.declare_dram_parameter("mtb", [P, P], dt16, isOutput=False)
    out_d = nc.declare_dram_parameter("out", [rows, width], F32, isOutput=True)

    with tile.TileContext(nc) as tc:
        with (
            tc.tile_pool(name="consts", bufs=1) as cpool,
            tc.tile_pool(name="xin", bufs=nrt + 4) as xpool,
            tc.tile_pool(name="y1t", bufs=nwt + 4) as y1tpool,
            tc.tile_pool(name="outp", bufs=4) as outpool,
            tc.tile_pool(name="psA", bufs=4, space="PSUM") as psA,
            tc.tile_pool(name="psB", bufs=4, space="PSUM") as psB,
        ):
            cb = _CopyBalancer(nc)
            mtb_sb = cpool.tile([P, P], dt16)
            nc.sync.dma_start(mtb_sb[:], mtb_d[:])

            for img in range(n_img):
                r0 = img * img_h

                xts = []
                for rt in range(nrt):
                    xt = xpool.tile([P, width], dt16)
                    # SWDGE cast f32 -> f16 during the load; halves ramp faster
                    for h in range(2):
                        cols = slice(h * width // 2, (h + 1) * width // 2)
                        nc.gpsimd.dma_start(
                            xt[:, cols],
                            x_d[r0 + rt * P : r0 + (rt + 1) * P, cols],
                        )
                    xts.append(xt)

                # pass 1: y1t[wt][:, rt*128:+128] = (M @ x_chunk)^T
                y1ts = []
                for wt in range(nwt):
                    y1t = y1tpool.tile([P, img_h], dt16)
                    for half in range(nrt // 4):
                        ps = psA.tile([P, 512], F32)
                        for q in range(4):
                            rt = half * 4 + q
                            nc.tensor.matmul(
                                ps[:, q * P : (q + 1) * P],
                                xts[rt][:, wt * P : (wt + 1) * P],
                                mtb_sb[:],
                            )
                        cb.copy(y1t[:, half * 512 : (half + 1) * 512], ps[:])
                    y1ts.append(y1t)

                # pass 2: out[rt][:, wt*128:+128] = (M @ y1t_chunk)^T = final
                for rt in range(nrt):
                    out_sb = outpool.tile([P, width], F32)
                    for half in range(nwt // 4):
                        ps = psB.tile([P, 512], F32)
                        for q in range(4):
                            wt = half * 4 + q
                            nc.tensor.matmul(
                                ps[:, q * P : (q + 1) * P],
                                y1ts[wt][:, rt * P : (rt + 1) * P],
                                mtb_sb[:],
                            )
                        cb.copy(out_sb[:, half * 512 : (half + 1) * 512], ps[:])
                        # store each 512-col half as soon as it lands
                        nc.sync.dma_start(
                            out_d[
                                r0 + rt * P : r0 + (rt + 1) * P,
                                half * 512 : (half + 1) * 512,
                            ],
                            out_sb[:, half * 512 : (half + 1) * 512],
                        )

    nc.compile()
    return nc


def build_v2(n_img: int, img_h: int, width: int, dt16=FP16, psum_fd=512,
             act_of_4=2, ps_bufs=8, store_eng="scalar", x_bufs=None):
    """fp16 DRAM in/out (host casts both ways): halves HBM traffic vs f32.

    HWDGE loads on the sync ring, stores on the scalar ring (separate FIFOs).
    Matmuls accumulate f32 into PSUM tiles of [128, psum_fd]; the PSUM->SBUF
    copy casts to fp16 and is split between ACT and DVE (act_of_4 of 4).
    """
    rows = n_img * img_h
    nrt, nwt = img_h // P, width // P
    mm_per_ps = psum_fd // P

    nc = bacc.Bacc("TRN2", target_bir_lowering=False, debug=False)
    x_d = nc.declare_dram_parameter("x", [rows, width], dt16, isOutput=False)
    mtb_d = nc.declare_dram_parameter("mtb", [P, P], dt16, isOutput=False)
    out_d = nc.declare_dram_parameter("out", [rows, width], dt16, isOutput=True)

    if x_bufs is None:
        x_bufs = nrt + 4

    with tile.TileContext(nc) as tc:
        with (
            tc.tile_pool(name="consts", bufs=1) as cpool,
            tc.tile_pool(name="xin", bufs=x_bufs) as xpool,
            tc.tile_pool(name="y1t", bufs=nwt + 4) as y1tpool,
            tc.tile_pool(name="outp", bufs=6) as outpool,
            tc.tile_pool(name="ps", bufs=ps_bufs, space="PSUM") as psp,
        ):
            store_engine = getattr(nc, store_eng)
            copy_i = 0

            def cb_copy(out, in_):
                nonlocal copy_i
                if copy_i % 4 < act_of_4:
                    nc.scalar.copy(out, in_)
                else:
                    nc.vector.tensor_copy(out, in_)
                copy_i += 1

            mtb_sb = cpool.tile([P, P], dt16)
            nc.sync.dma_start(mtb_sb[:], mtb_d[:])

            for img in range(n_img):
                r0 = img * img_h

                xts = []
                for rt in range(nrt):
                    xt = xpool.tile([P, width], dt16)
                    nc.sync.dma_start(
                        xt[:], x_d[r0 + rt * P : r0 + (rt + 1) * P, :]
                    )
                    xts.append(xt)

                # pass 1: y1t[wt][:, rt*128:+128] = (M @ x_chunk)^T
                y1ts = []
                for wt in range(nwt):
                    y1t = y1tpool.tile([P, img_h], dt16)
                    for g in range(nrt // mm_per_ps):
                        ps = psp.tile([P, psum_fd], F32)
                        for q in range(mm_per_ps):
                            rt = g * mm_per_ps + q
                            nc.tensor.matmul(
                                ps[:, q * P : (q + 1) * P],
                                xts[rt][:, wt * P : (wt + 1) * P],
                                mtb_sb[:],
                            )
                        cb_copy(
                            y1t[:, g * psum_fd : (g + 1) * psum_fd], ps[:]
                        )
                    y1ts.append(y1t)

                # pass 2: out[rt][:, wt*128:+128] = (M @ y1t_chunk)^T = final
                for rt in range(nrt):
                    out_sb = outpool.tile([P, width], dt16)
                    for g in range(nwt // mm_per_ps):
                        ps = psp.tile([P, psum_fd], F32)
                        for q in range(mm_per_ps):
                            wt = g * mm_per_ps + q
                            nc.tensor.matmul(
                                ps[:, q * P : (q + 1) * P],
                                y1ts[wt][:, rt * P : (rt + 1) * P],
                                mtb_sb[:],
                            )
                        cb_copy(
                            out_sb[:, g * psum_fd : (g + 1) * psum_fd], ps[:]
                        )
                    store_engine.dma_start(
                        out_d[r0 + rt * P : r0 + (rt + 1) * P, :], out_sb[:]
                    )

    nc.compile()
    return nc


def build_v4(n_img: int, img_h: int, width: int, dt16=FP16, psum_fd=1024,
             act_of_4=2, ps_bufs=4, x_bufs=20, y1t_bufs=18, out_bufs=6,
             warm_mm=48, load_gran=1, store_gran=1):
    """v3 + software pipelining: pass2 of image i-1 interleaves with pass1 of
    image i at PSUM-group granularity, so pass2 stationaries (y1t copies)
    have a full image-period of slack. A dummy-matmul warmup burst runs
    during the first image's load so HAM un-throttles before real work.
    """
    rows = n_img * img_h
    nrt, nwt = img_h // P, width // P
    mm_per_ps = psum_fd // P

    nc = bacc.Bacc("TRN2", target_bir_lowering=False, debug=False)
    x_d = nc.declare_dram_parameter("x", [rows, width], dt16, isOutput=False)
    mtb_d = nc.declare_dram_parameter("mtb", [P, P], dt16, isOutput=False)
    out_d = nc.declare_dram_parameter("out", [rows, width], dt16, isOutput=True)

    with tile.TileContext(nc) as tc:
        with (
            tc.tile_pool(name="consts", bufs=1) as cpool,
            tc.tile_pool(name="xin", bufs=x_bufs) as xpool,
            tc.tile_pool(name="y1t", bufs=y1t_bufs) as y1tpool,
            tc.tile_pool(name="outp", bufs=out_bufs) as outpool,
            tc.tile_pool(name="ps", bufs=ps_bufs, space="PSUM") as psp,
        ):
            copy_i = 0

            def cb_copy(out, in_):
                nonlocal copy_i
                if copy_i % 4 < act_of_4:
                    nc.scalar.copy(out, in_)
                else:
                    nc.vector.tensor_copy(out, in_)
                copy_i += 1

            mtb_sb = cpool.tile([P, P], dt16)
            nc.sync.dma_start(mtb_sb[:], mtb_d[:])

            xts_of = {}
            y1ts_of = {}

            def load_img(i):
                # granule tiles of load_gran row-tiles; per-partition runs
                # stay 2KB-contiguous, batched into one descriptor set
                r0 = i * img_h
                xg = []
                for g in range(nrt // load_gran):
                    xt = xpool.tile([P, load_gran, width], dt16)
                    src = x_d[
                        r0 + g * load_gran * P : r0 + (g + 1) * load_gran * P, :
                    ].rearrange("(t p) w -> p t w", p=P)
                    nc.sync.dma_start(xt[:], src)
                    xg.append(xt)
                xts_of[i] = xg

            def x_chunk(i, rt, wt):
                xt = xts_of[i][rt // load_gran]
                return xt[:, rt % load_gran, wt * P : (wt + 1) * P]

            def p1_group(i, wt):
                # y1t(i)[wt][:, rt*128:+128] = (M @ x_chunk)^T for all rt
                y1t = y1tpool.tile([P, img_h], dt16)
                for g in range(nrt // mm_per_ps):
                    ps = psp.tile([P, psum_fd], F32)
                    for q in range(mm_per_ps):
                        rt = g * mm_per_ps + q
                        nc.tensor.matmul(
                            ps[:, q * P : (q + 1) * P],
                            x_chunk(i, rt, wt),
                            mtb_sb[:],
                        )
                    cb_copy(y1t[:, g * psum_fd : (g + 1) * psum_fd], ps[:])
                y1ts_of.setdefault(i, []).append(y1t)

            out_cur = {}

            def p2_group(i, rt, half=None):
                # out(i)[rt][:, wt*128:+128] = (M @ y1t_chunk)^T
                # half=0/1: emit only the wt-half (4 matmuls + 1 copy); half 0
                # needs only y1t[0..3], so it can start before pass1 finishes
                r0 = i * img_h
                y1ts = y1ts_of[i]
                seg = rt % store_gran
                pair = rt - seg
                if pair not in out_cur:
                    out_cur[pair] = outpool.tile(
                        [P, store_gran, width], dt16, name="out_sb"
                    )
                out_sb = out_cur[pair]
                if half is None:
                    ps = psp.tile([P, psum_fd], F32)
                    for q in range(mm_per_ps):
                        nc.tensor.matmul(
                            ps[:, q * P : (q + 1) * P],
                            y1ts[q][:, rt * P : (rt + 1) * P],
                            mtb_sb[:],
                        )
                    cb_copy(out_sb[:, seg, :], ps[:])
                else:
                    ps = psp.tile([P, psum_fd], F32)
                    fd = psum_fd // 2
                    for q in range(fd // P):
                        wt = half * (fd // P) + q
                        nc.tensor.matmul(
                            ps[:, q * P : (q + 1) * P],
                            y1ts[wt][:, rt * P : (rt + 1) * P],
                            mtb_sb[:],
                        )
                    cb_copy(
                        out_sb[:, seg, half * fd : (half + 1) * fd], ps[:, :fd]
                    )
                if (half is None or half == 1) and seg == store_gran - 1:
                    del out_cur[pair]
                    dst = out_d[
                        r0 + pair * P : r0 + (pair + store_gran) * P, :
                    ].rearrange("(t p) w -> p t w", p=P)
                    nc.gpsimd.dma_start(dst, out_sb[:])

            # HAM warmup: dummy matmuls on the constant while image 0 loads.
            load_img(0)
            for w in range(warm_mm // mm_per_ps):
                ps = psp.tile([P, psum_fd], F32)
                for q in range(mm_per_ps):
                    nc.tensor.matmul(
                        ps[:, q * P : (q + 1) * P], mtb_sb[:], mtb_sb[:]
                    )

            for i in range(n_img):
                if i + 1 < n_img:
                    load_img(i + 1)
                for g in range(nwt):
                    p1_group(i, g)
                    if i > 0:
                        p2_group(i - 1, g)
            for g in range(nwt):
                p2_group(n_img - 1, g)

    nc.compile()
    return nc


def build_hybrid(n_img: int, img_h: int, width: int):
    rows = n_img * img_h
    nrt, nwt = img_h // P, width // P
    assert nrt % 4 == 0 and nwt % 4 == 0
    MMW = 512

    nc = bacc.Bacc("TRN2", target_bir_lowering=False, debug=False)
    x_d = nc.declare_dram_parameter("x", [rows, width], F32R, isOutput=False)
    mt_d = nc.declare_dram_parameter("mt", [P, P], F32R, isOutput=False)
    mtb_d = nc.declare_dram_parameter("mtb", [P, P], BF16, isOutput=False)
    identb_d = nc.declare_dram_parameter("identb", [P, P], BF16, isOutput=False)
    out_d = nc.declare_dram_parameter("out", [rows, width], F32, isOutput=True)

    with tile.TileContext(nc) as tc:
        with (
            tc.tile_pool(name="consts", bufs=1) as cpool,
            tc.tile_pool(name="xin", bufs=6) as xpool,
            tc.tile_pool(name="y1", bufs=nrt + 2) as y1pool,
            tc.tile_pool(name="y1t", bufs=nwt + 4) as y1tpool,
            tc.tile_pool(name="outp", bufs=4) as outpool,
            tc.tile_pool(name="psV", bufs=3, space="PSUM") as psV,
            tc.tile_pool(name="psT", bufs=3, space="PSUM") as psT,
            tc.tile_pool(name="psH", bufs=2, space="PSUM") as psH,
        ):
            cb = _CopyBalancer(nc)
            mt_sb = cpool.tile([P, P], F32R)
            mtb_sb = cpool.tile([P, P], BF16)
            identb = cpool.tile([P, P], BF16)
            nc.sync.dma_start(mt_sb[:], mt_d[:])
            nc.sync.dma_start(mtb_sb[:], mtb_d[:])
            nc.sync.dma_start(identb[:], identb_d[:])

            for img in range(n_img):
                r0 = img * img_h

                # V-pass: f32r stream, round to bf16 on the PSUM->SBUF copy
                y1s = []
                for rt in range(nrt):
                    xt = xpool.tile([P, width], F32R)
                    nc.sync.dma_start(
                        xt[:], x_d[r0 + rt * P : r0 + (rt + 1) * P, :]
                    )
                    y1 = y1pool.tile([P, width], BF16)
                    for c in range(width // MMW):
                        ps = psV.tile([P, MMW], F32)
                        nc.tensor.matmul(
                            ps[:], mt_sb[:], xt[:, c * MMW : (c + 1) * MMW]
                        )
                        cb.copy(y1[:, c * MMW : (c + 1) * MMW], ps[:])
                    y1s.append(y1)

                # T-pass: bf16 PE transposes, 8 per PSUM bank
                y1ts = []
                for wt in range(nwt):
                    y1t = y1tpool.tile([P, img_h], BF16)
                    pst = psT.tile([P, img_h], BF16)
                    for rt in range(nrt):
                        nc.tensor.transpose(
                            pst[:, rt * P : (rt + 1) * P],
                            y1s[rt][:, wt * P : (wt + 1) * P],
                            identb[:],
                        )
                    cb.copy(y1t[:], pst[:])
                    y1ts.append(y1t)

                # fused H-pass: out chunk = (y1t_chunk)^T @ M^T  (H-major)
                for rt in range(nrt):
                    out_sb = outpool.tile([P, width], F32)
                    for half in range(nwt // 4):
                        ps = psH.tile([P, 512], F32)
                        for q in range(4):
                            wt = half * 4 + q
                            nc.tensor.matmul(
                                ps[:, q * P : (q + 1) * P],
                                y1ts[wt][:, rt * P : (rt + 1) * P],
                                mtb_sb[:],
                            )
                        cb.copy(out_sb[:, half * 512 : (half + 1) * 512], ps[:])
                    nc.sync.dma_start(
                        out_d[r0 + rt * P : r0 + (rt + 1) * P, :], out_sb[:]
                    )

    nc.compile()
    return nc


def build_nc(n_img, img_h, width, mode=MODE):
    if mode == "fused_bf16":
        return build_fused_bf16(n_img, img_h, width, BF16)
    if mode == "fused_fp16":
        return build_fused_bf16(n_img, img_h, width, FP16)
    if mode == "hybrid":
        return build_hybrid(n_img, img_h, width)
    if mode == "v2":
        return build_v2(n_img, img_h, width)
    if mode == "v3":
        return build_v2(n_img, img_h, width, psum_fd=1024, ps_bufs=4,
                        store_eng="gpsimd", x_bufs=20)
    if mode == "v4":
        return build_v4(n_img, img_h, width)
    if mode == "v6":
        return build_v4(n_img, img_h, width, load_gran=4, store_gran=2,
                        x_bufs=5, out_bufs=4)
    raise ValueError(mode)


def make_mt(A: np.ndarray) -> np.ndarray:
    """M^T where M = kron(I_{128/8}, A)."""
    M = np.kron(np.eye(P // BLOCK, dtype=np.float32), A.astype(np.float32))
    return np.ascontiguousarray(M.T)


def make_inputs(mode, x_core, A):
    mt = make_mt(A)
    if mode == "fused_bf16":
        return {"x": x_core, "mtb": mt.astype(ml_dtypes.bfloat16)}
    if mode == "fused_fp16":
        return {"x": x_core, "mtb": mt.astype(np.float16)}
    if mode[0] == "v" and mode[1].isdigit() and int(mode[1]) >= 2:
        return {"x": x_core.astype(np.float16), "mtb": mt.astype(np.float16)}
    if mode == "hybrid":
        return {
            "x": x_core,
            "mt": mt,
            "mtb": mt.astype(ml_dtypes.bfloat16),
            "identb": np.eye(P, dtype=ml_dtypes.bfloat16),
        }
    raise ValueError(mode)


_NC_CACHE = {}


def _get_nc(key, *args, **kwargs):
    if key not in _NC_CACHE:
        _NC_CACHE[key] = build_nc(*args, **kwargs)
    return _NC_CACHE[key]


def kernel(x: np.ndarray, A: np.ndarray) -> np.ndarray:
    x = np.asarray(x, dtype=np.float32)
    A = np.asarray(A, dtype=np.float32)
    N, C, H, W = x.shape
    assert (N, C, H, W) == (FULL_N, FULL_C, FULL_H, FULL_W), x.shape
    per = N // N_CORES

    nc = _get_nc(("full", MODE), per * C, H, W, MODE)

    in_maps = [
        make_inputs(
            MODE,
            np.ascontiguousarray(x[c * per : (c + 1) * per].reshape(per * C * H, W)),
            A,
        )
        for c in range(N_CORES)
    ]
    def dc_ok(outs):
        # DC coeff of block (0,0) must equal mean*8 of the 8x8 input block;
        # catches transient device faults that corrupt output silently
        for n in range(0, N, 7):
            dc = float(x[n, 0, :8, :8].sum()) / 8.0
            if abs(float(outs[n // per][n % per, 0, 0, 0]) - dc) > 0.05 + 0.02 * abs(dc):
                return False
        return True

    last_err = None
    for _attempt in range(3):
        try:
            res = run_bass_kernel_spmd(nc, in_maps, list(range(N_CORES)))
        except Exception as e:  # transient NRT device faults: retry
            last_err = e
            continue
        outs = [
            res.results[c]["out"].astype(np.float32, copy=False).reshape(per, C, H, W)
            for c in range(N_CORES)
        ]
        if dc_ok(outs):
            return np.concatenate(outs, axis=0)
        last_err = RuntimeError("DC self-check failed (corrupt output)")
    raise last_err

